# revision 32
# baseline (speedup 1.0000x reference)
"""Conv2d(128->256, 3x3, pad 1) with LoRA (rank 8) — Trainium2 Bass kernel.

Strategy:
  - Data-parallel over batch: 16 images -> 2 per core x 8 cores; weights
    replicated.
  - LoRA folds into the conv weight on the host (conv is linear in weights,
    and W_eff = W + (alpha/rank) * (B @ A) is 0.3 MFLOP vs the conv's
    4.8 GFLOP): the device kernel is a pure 3x3 conv with a bias.
  - The conv = 9 shifted matmuls accumulating in PSUM:
        out[co, pix] += W_eff[co, :, kh, kw]^T @ x_shift[ci, pix]
    with K = C_IN = 128 (partition dim), M = 128 (co block), N = 512
    (8 image rows x 64 cols) in bf16 (full PE rate, weight loads hidden).
  - DMA cost here is dominated by per-partition-row overhead (~21ns/row),
    not bytes: every transfer is shaped to maximize bytes/row and minimize
    row count. x and W_eff ship bf16 as ONE transfer each (128 rows);
    outputs go out as row-group PAIRS (128 rows x 4KB); the final transfers
    are partition-split across queues so the last-write latency is short.
  - The PE p-state ramp needs ~3us of CONTINUOUS busy before full clock
    (and resets on idle gaps): warm-up matmuls bridge the input DMA window
    and the conv stream is kept gap-free after that.
"""

import numpy as np

import concourse.bass as bass
import concourse.tile as tile
from concourse.tile import add_dep_helper
from concourse import bacc, mybir
from concourse.bass_utils import run_bass_kernel_spmd

N_CORES = 8
B, C_IN, H, W_DIM = 16, 128, 64, 64
C_OUT = 256
RANK = 8
SCALING = 2.0  # alpha/rank = 16/8
HP, WP = H + 2, W_DIM + 2  # zero-padded image dims
B_LOC = B // N_CORES  # images per core
NPIX = H * W_DIM  # 4096
ROWS_PER_TILE = 8  # output rows per matmul group -> N = 8*64 = 512
N_RG = H // ROWS_PER_TILE  # 8 row groups
WCOLS = 2 + 9 * C_OUT  # [bias (2 cols) | weff (9*256 cols)]

F32 = mybir.dt.float32
BF16 = mybir.dt.bfloat16
IDENT = mybir.ActivationFunctionType.Identity


def _build_nc():
    nc = bacc.Bacc(
        "TRN2",
        target_bir_lowering=False,
        debug=False,
        num_devices=N_CORES,
    )

    xp = nc.dram_tensor("xp", [B_LOC, C_IN, HP * WP], BF16, kind="ExternalInput").ap()
    wt = nc.dram_tensor("wt", [C_IN, WCOLS], BF16, kind="ExternalInput").ap()
    out = nc.dram_tensor("out", [B_LOC, C_OUT, NPIX], F32, kind="ExternalOutput").ap()

    with tile.TileContext(nc) as tc:
        with (
            tc.tile_pool(name="persist", bufs=1) as persist,
            tc.tile_pool(name="outp", bufs=4) as outp,
            tc.tile_pool(name="psum", bufs=7, space="PSUM") as psum,
        ):
            # --- persistent SBUF tiles -------------------------------------
            x_sb = [
                persist.tile([C_IN, HP * WP], BF16, name=f"x_sb{i}")
                for i in range(B_LOC)
            ]
            weff = persist.tile([C_IN, WCOLS], BF16, name="weff")
            b32 = persist.tile([128, 2], F32, name="b32")
            warm_sb = persist.tile([128, 512], F32, name="warm_sb")

            # --- PE warm-up ------------------------------------------------
            # fp32 matmuls (4 cycles/col) bridge the ~6us until weff is fully
            # resident, so the conv stream starts at full clock and never
            # stalls (a stall would reset the p-state ramp to 1.2GHz).
            nc.gpsimd.memset(warm_sb[:], 0.0)
            warm_ps = psum.tile([128, 512], F32, tag="warm", bufs=1, name="warm_ps")
            for ncols in (512, 512, 512):
                nc.tensor.matmul(
                    warm_ps[:, :ncols],
                    warm_sb[:, :128],
                    warm_sb[:, :ncols],
                    start=True,
                    stop=True,
                )

            # --- input DMAs ------------------------------------------------
            # Each HW queue sustains only ~100GB/s per transfer, so the
            # startup-critical bytes are chunked by need-time across both
            # queues: x0 thirds on sync, weff (bias+k0-3 / k4-6 / k7-8) on
            # scalar. Image 1 rides gpsimd, held back behind an early conv
            # tile so it doesn't contend during startup.
            qs = [nc.sync, nc.scalar]
            # First-transfer latency per queue is ~2.3us, then ~300+GB/s with
            # ~0.7us between transfers: weff goes as ONE transfer on scalar
            # (it gates the conv start) while x0 streams in thirds on sync.
            wh = 2 + 1152  # bias + all of cb0's k-slices
            nc.scalar.dma_start(weff[:, :wh], wt[:, :wh])
            nc.scalar.dma_start(weff[:, wh:], wt[:, wh:])
            xth = 1452  # x0 thirds: 22 image rows each
            for c in range(3):
                nc.sync.dma_start(
                    x_sb[0][:, c * xth : (c + 1) * xth],
                    xp[0, :, c * xth : (c + 1) * xth],
                )
            x1_dma = nc.gpsimd.dma_start(x_sb[1][:], xp[1])
            # bias shipped as the first 2 bf16 cols of wt; widen once for the
            # f32-only DVE/ACT bias ports
            nc.vector.tensor_copy(b32[:], weff[:, 0:2])

            # --- the conv: 9 accumulating shift-matmuls per output tile ----
            def conv_tile(x_r, cb, h0, nrows, ps):
                mm = None
                for k in range(9):
                    dh, dw = k // 3 - 1, k % 3 - 1
                    rhs = x_r[
                        :,
                        h0 + 1 + dh : h0 + 1 + dh + nrows,
                        1 + dw : 65 + dw,
                    ]
                    co0 = 2 + cb * 1152 + k * 128
                    lhsT = weff[:, co0 : co0 + 128]
                    mm = nc.tensor.matmul(
                        ps[:], lhsT, rhs, start=(k == 0), stop=(k == 8)
                    )
                return mm

            def bias_act(dst_ap, src_ap, cb):
                nc.scalar.activation(dst_ap, src_ap, IDENT, bias=b32[:, cb : cb + 1])

            def bias_dve(dst_ap, src_ap, cb):
                nc.vector.tensor_scalar_add(dst_ap, src_ap, b32[:, cb : cb + 1])

            for img in range(B_LOC):
                x_r = x_sb[img][:].rearrange("p (h w) -> p h w", w=WP)
                for cb in range(2):
                    for rg in range(N_RG):
                        ti = (img * 2 + cb) * N_RG + rg
                        h0 = rg * ROWS_PER_TILE
                        dst = out[
                            img, cb * 128 : (cb + 1) * 128, rg * 512 : (rg + 1) * 512
                        ]
                        if ti < 31:
                            ps = psum.tile([128, 512], F32, tag="ps", name=f"ps{ti}")
                            mm = conv_tile(x_r, cb, h0, ROWS_PER_TILE, ps)
                            if ti == 0:
                                # release image-1's DMA only once the startup
                                # transfers are out of the way
                                add_dep_helper(
                                    x1_dma.ins,
                                    mm.ins,
                                    reason="defer x1 traffic past startup",
                                )
                            o = outp.tile([128, 512], F32, tag="o", name=f"o{ti}")
                            if ti >= 28:
                                # near the tail, split drains across both
                                # engines so the last PSUM->SBUF steps are
                                # short
                                bias_act(o[:, :256], ps[:, :256], cb)
                                bias_dve(o[:, 256:], ps[:, 256:], cb)
                            elif ti % 2 == 0:
                                bias_act(o[:], ps[:], cb)
                            else:
                                bias_dve(o[:], ps[:], cb)
                            # every output tile leaves as two 64-row
                            # transfers, one per HW queue (short completions)
                            qs[0].dma_start(dst[0:64, :], o[0:64, :])
                            qs[1].dma_start(dst[64:128, :], o[64:128, :])
                        else:
                            # Final row group: two 256-col sub-tiles with
                            # split drains and partition-split DMAs so the
                            # last matmul -> last HBM write path is short.
                            for hf in range(2):
                                ps2 = psum.tile(
                                    [128, 256], F32, tag="ps", name=f"ps_t31_{hf}"
                                )
                                conv_tile(x_r, cb, h0 + hf * 4, 4, ps2)
                                o2 = outp.tile(
                                    [128, 256], F32, tag="o", name=f"o31_{hf}"
                                )
                                bias_act(o2[:, :128], ps2[:, :128], cb)
                                bias_dve(o2[:, 128:], ps2[:, 128:], cb)
                                dsth = dst[:, hf * 256 : (hf + 1) * 256]
                                qs[0].dma_start(dsth[0:64, :], o2[0:64, :])
                                qs[1].dma_start(dsth[64:128, :], o2[64:128, :])

    nc.compile()
    return nc


_NC_CACHE = None


def _get_nc():
    global _NC_CACHE
    if _NC_CACHE is None:
        _NC_CACHE = _build_nc()
    return _NC_CACHE


def _host_prep(x, W, b, lora_A, lora_B):
    """Host prep: fold LoRA into the conv weight, pad, transpose, bf16."""
    bf16 = mybir.dt.np(BF16)
    x = np.asarray(x, dtype=np.float32)
    xp_all = np.zeros((B, C_IN, HP, WP), dtype=bf16)
    xp_all[:, :, 1 : H + 1, 1 : W_DIM + 1] = x.astype(bf16)
    xp_all = xp_all.reshape(B, C_IN, HP * WP)

    # W_eff = W + (alpha/rank) * (B @ A), then [co, ci, k] -> [ci, k, co]
    weff = (
        np.asarray(W, dtype=np.float32).reshape(C_OUT, C_IN * 9)
        + SCALING
        * (np.asarray(lora_B, np.float32) @ np.asarray(lora_A, np.float32))
    )
    # cb-major: wk[ci, cb*1152 + k*128 + j] = W_eff[cb*128+j, ci, k]
    wk = np.ascontiguousarray(
        weff.reshape(2, 128, C_IN, 9).transpose(2, 0, 3, 1)
    ).reshape(C_IN, 9 * C_OUT)
    # wt = [bias (2 cols: bv[p, cb] = b[cb*128 + p]) | weff], all bf16
    bv = np.asarray(b, dtype=np.float32).reshape(2, 128).T
    wt = np.ascontiguousarray(np.concatenate([bv, wk], axis=1).astype(bf16))
    return xp_all, wt


def run(x, W, b, lora_A, lora_B, trace=False):
    """Run the kernel on 8 cores; returns (full_output, BassKernelResults)."""
    xp_all, wt = _host_prep(x, W, b, lora_A, lora_B)
    nc = _get_nc()
    in_maps = []
    for c in range(N_CORES):
        in_maps.append(
            {
                "xp": np.ascontiguousarray(xp_all[c * B_LOC : (c + 1) * B_LOC]),
                "wt": wt,
            }
        )
    res = run_bass_kernel_spmd(
        nc, in_maps, core_ids=list(range(N_CORES)), trace=trace
    )
    out = np.concatenate([r["out"] for r in res.results], axis=0)
    return out.reshape(B, C_OUT, H, W_DIM), res


def kernel(x, W, b, lora_A, lora_B):
    out, _ = run(x, W, b, lora_A, lora_B, trace=False)
    return out
